# revision 12
# baseline (speedup 1.0000x reference)
"""LocationSensitiveAttention Trainium2 kernel.

Data-parallel over batch: 64 batches -> 8 cores x 8 batches.

Per core, per batch b:
  - value[b] [2048,256] loaded natural layout as [128p, 16n, 256d] (one DMA)
  - PE-transposes -> valT [128d x2, 2048t]
  - conv1d folded into a single matmul: X[ck, t] = xpad[b, c, t+k] windows
    (built with 2 overlapping-window DMAs from host-padded input),
    WcombT = (Wloc @ Wconv.reshape(32,62)).T  folded on host (weights only)
  - energyT psum [128a, 512t] chunks = WvT.T@valT + WcombT.T@X
  - tanh via ACT with bias = (Wq@query[b]+bias)[a] per-partition (computed on
    device once per core)
  - scores chunk = Wa.T @ energyT -> psum_scores[b, chunk]
Then once per core: masked softmax over t for all 8 batches [8,2048],
alignment DMA out, PE-transpose alignment, context = align.T @ value
accumulated over 16 t-tiles per batch.
"""

import numpy as np

import concourse.bacc as bacc
import concourse.bass as bass
import concourse.mybir as mybir
from concourse.bass_utils import run_bass_kernel_spmd
from concourse.masks import make_identity
from concourse.tile import TileContext

B, T = 64, 2048
REC_HID, VAL_DIM, ATT_DIM = 512, 256, 128
FILT, KSZ, PAD = 32, 31, 15
NCORES = 8
BL = B // NCORES          # 8 batches per core
NT = T // 128             # 16 value t-tiles per batch
NCHUNK = 4                # energy chunks per batch
CH = T // NCHUNK          # 512
CK = 2 * KSZ              # 62 im2col rows
TP = T + 2 * PAD          # padded time length
FP32 = mybir.dt.float32
AF = mybir.ActivationFunctionType

_CACHE = {}


def _build_nc():
    nc = bacc.Bacc("TRN2", target_bir_lowering=False, debug=False,
                   num_devices=NCORES)

    value = nc.dram_tensor("value", [BL, T, VAL_DIM], FP32, kind="ExternalInput")
    xpad = nc.dram_tensor("xpad", [BL, 2, TP], FP32, kind="ExternalInput")
    queryl = nc.dram_tensor("queryl", [BL, REC_HID], FP32, kind="ExternalInput")
    maskl = nc.dram_tensor("maskl", [BL, T], mybir.dt.uint8, kind="ExternalInput")
    WvT = nc.dram_tensor("WvT", [VAL_DIM, ATT_DIM], FP32, kind="ExternalInput")
    WqT = nc.dram_tensor("WqT", [REC_HID, ATT_DIM], FP32, kind="ExternalInput")
    WcT = nc.dram_tensor("WcT", [CK, ATT_DIM], FP32, kind="ExternalInput")
    Wa = nc.dram_tensor("Wa", [ATT_DIM, 1], FP32, kind="ExternalInput")
    WaEmb = nc.dram_tensor("WaEmb", [ATT_DIM, BL * BL], FP32,
                           kind="ExternalInput")
    biasq = nc.dram_tensor("biasq", [ATT_DIM, 1], FP32, kind="ExternalInput")
    out_ctx = nc.dram_tensor("out_ctx", [BL, VAL_DIM], FP32, kind="ExternalOutput")
    out_align = nc.dram_tensor("out_align", [BL, T], FP32, kind="ExternalOutput")

    with TileContext(nc) as tc:
        from contextlib import ExitStack
        with ExitStack() as ctx:
            wpool = ctx.enter_context(tc.tile_pool(name="weights", bufs=1))
            valpool = ctx.enter_context(tc.tile_pool(name="val", bufs=1))
            vtpool = ctx.enter_context(tc.tile_pool(name="valT", bufs=1))
            xpool = ctx.enter_context(tc.tile_pool(name="xwin", bufs=2))
            etpool = ctx.enter_context(tc.tile_pool(name="energyT", bufs=2))
            smpool = ctx.enter_context(tc.tile_pool(name="softmax", bufs=1))
            ptpool = ctx.enter_context(tc.tile_pool(name="ptrans", bufs=2, space="PSUM"))
            pepool = ctx.enter_context(tc.tile_pool(name="penergy", bufs=2, space="PSUM"))
            pspool = ctx.enter_context(tc.tile_pool(name="pscores", bufs=1, space="PSUM"))

            # ---- constants / weights in SBUF ----
            ident = wpool.tile([128, 128], FP32)
            make_identity(nc, ident)
            wvt_sb = wpool.tile([128, 2, ATT_DIM], FP32)
            nc.sync.dma_start(
                out=wvt_sb, in_=WvT.ap().rearrange("(h p) a -> p h a", p=128))
            wqt_sb = wpool.tile([128, 4, ATT_DIM], FP32)
            nc.sync.dma_start(
                out=wqt_sb, in_=WqT.ap().rearrange("(h p) a -> p h a", p=128))
            wct_sb = wpool.tile([CK, ATT_DIM], FP32)
            nc.sync.dma_start(out=wct_sb, in_=WcT.ap())
            wa_sb = wpool.tile([ATT_DIM, 1], FP32)
            nc.sync.dma_start(out=wa_sb, in_=Wa.ap())
            waemb_sb = wpool.tile([ATT_DIM, BL * BL], FP32)
            nc.sync.dma_start(out=waemb_sb, in_=WaEmb.ap())
            bias_sb = wpool.tile([ATT_DIM, 1], FP32)
            nc.sync.dma_start(out=bias_sb, in_=biasq.ap())

            # ---- q projection: qb_all[a, b] = Wq @ query[b] + bias ----
            query_sb = wpool.tile([BL, REC_HID], FP32)
            nc.sync.dma_start(out=query_sb, in_=queryl.ap())
            queryT = wpool.tile([128, 4, BL], FP32)
            for k in range(4):
                pt = ptpool.tile([128, 128], FP32, tag="pt")
                nc.tensor.transpose(
                    pt[:, :BL], query_sb[:, k * 128:(k + 1) * 128],
                    ident[:BL, :BL])
                nc.vector.tensor_copy(queryT[:, k, :], pt[:, :BL])
            qb_psum = pepool.tile([128, BL], FP32, tag="pe")
            for k in range(4):
                nc.tensor.matmul(qb_psum, wqt_sb[:, k, :], queryT[:, k, :],
                                 start=(k == 0), stop=(k == 3))
            qb_all = wpool.tile([128, BL], FP32)
            nc.scalar.activation(qb_all, qb_psum, AF.Identity,
                                 bias=bias_sb[:, 0:1], scale=1.0)

            # persistent scores psum [8, 2048] = 4 banks
            psum_scores = pspool.tile([BL, T], FP32)

            val_tiles = []
            for b in range(BL):
                # ---- load value[b] natural [128, 16, 256] ----
                val_b = valpool.tile([128, NT, VAL_DIM], FP32, tag=f"val{b}")
                nc.sync.dma_start(
                    out=val_b,
                    in_=value.ap()[b].rearrange("(n p) d -> p n d", p=128))
                val_tiles.append(val_b)

                # ---- im2col windows X[ck, t] via overlapping DMA ----
                x_b = xpool.tile([CK, T], FP32, tag="X")
                for c in range(2):
                    src = bass.AP(xpad, (b * 2 + c) * TP, [[1, KSZ], [1, T]])
                    nc.sync.dma_start(out=x_b[c * KSZ:(c + 1) * KSZ, :], in_=src)

                # ---- transpose value -> valT [128d, 2 dh, 2048t] ----
                valT = vtpool.tile([128, 2, T], FP32, tag="valT")
                for n in range(NT):
                    for dh in range(2):
                        pt = ptpool.tile([128, 128], FP32, tag="pt")
                        nc.tensor.transpose(
                            pt, val_b[:, n, dh * 128:(dh + 1) * 128], ident)
                        dst = valT[:, dh, n * 128:(n + 1) * 128]
                        if (n * 2 + dh) % 2 == 0:
                            nc.vector.tensor_copy(dst, pt)
                        else:
                            nc.scalar.copy(dst, pt)

                # ---- energy chunks + scores ----
                for c in range(NCHUNK):
                    pe = pepool.tile([128, CH], FP32, tag="pe")
                    sl = slice(c * CH, (c + 1) * CH)
                    nc.tensor.matmul(pe, wvt_sb[:, 0, :], valT[:, 0, sl],
                                     start=True, stop=False)
                    nc.tensor.matmul(pe, wvt_sb[:, 1, :], valT[:, 1, sl],
                                     start=False, stop=False)
                    nc.tensor.matmul(pe, wct_sb, x_b[:, sl],
                                     start=False, stop=True)
                    et = etpool.tile([128, CH], FP32, tag="et")
                    nc.scalar.activation(et, pe, AF.Tanh,
                                         bias=qb_all[:, b:b + 1], scale=1.0)
                    # accumulate this batch's scores into row b of the shared
                    # [8, chunk] region: lhsT col j is Wa iff j == b, else 0
                    nc.tensor.matmul(psum_scores[:, sl],
                                     waemb_sb[:, b * BL:(b + 1) * BL], et,
                                     start=(b == 0), stop=(b == BL - 1))

            # ---- masked softmax over t for all 8 batches ----
            scores_sb = smpool.tile([BL, T], FP32)
            nc.vector.tensor_copy(scores_sb, psum_scores)
            mask_u8 = smpool.tile([BL, T], mybir.dt.uint8)
            nc.sync.dma_start(out=mask_u8, in_=maskl.ap())
            work = smpool.tile([BL, T], FP32)
            nc.vector.tensor_scalar_mul(work, mask_u8, -1e30)
            nc.vector.tensor_add(scores_sb, scores_sb, work)
            rmax_neg = smpool.tile([BL, 1], FP32)
            nc.vector.tensor_reduce(rmax_neg, scores_sb,
                                    axis=mybir.AxisListType.X,
                                    op=mybir.AluOpType.max, negate=True)
            ssum = smpool.tile([BL, 1], FP32)
            nc.scalar.activation(work, scores_sb, AF.Exp,
                                 bias=rmax_neg[:, 0:1], scale=1.0,
                                 accum_out=ssum[:, 0:1])
            rinv = smpool.tile([BL, 1], FP32)
            nc.vector.reciprocal(rinv, ssum)
            nc.vector.tensor_scalar_mul(work, work, rinv[:, 0:1])
            nc.sync.dma_start(out=out_align.ap(), in_=work)

            # ---- context = align.T @ value ----
            alignT = smpool.tile([128, NT, BL], FP32)
            for n in range(NT):
                pt = ptpool.tile([128, 128], FP32, tag="pt")
                nc.tensor.transpose(
                    pt[:, :BL], work[:, n * 128:(n + 1) * 128],
                    ident[:BL, :BL])
                nc.vector.tensor_copy(alignT[:, n, :], pt[:, :BL])
            ctx_sb = smpool.tile([1, BL, VAL_DIM], FP32)
            for b in range(BL):
                ctx_psum = pepool.tile([1, VAL_DIM], FP32, tag="pe")
                for n in range(NT):
                    nc.tensor.matmul(ctx_psum,
                                     alignT[:, n, b:b + 1],
                                     val_tiles[b][:, n, :],
                                     start=(n == 0), stop=(n == NT - 1))
                nc.vector.tensor_copy(ctx_sb[:, b, :], ctx_psum)
            nc.sync.dma_start(
                out=out_ctx.ap().rearrange("b d -> (b d)"),
                in_=ctx_sb.rearrange("p b d -> p (b d)"))

    nc.compile()
    return nc


def get_nc():
    if "nc" not in _CACHE:
        _CACHE["nc"] = _build_nc()
    return _CACHE["nc"]


def make_in_maps(query, value, last_alignment_weghts, mask, Wq, Wv, Wa, bias,
                 Wconv, Wloc):
    query = np.asarray(query, dtype=np.float32)
    value = np.asarray(value, dtype=np.float32)
    law = np.asarray(last_alignment_weghts, dtype=np.float32)
    mask_u8 = np.asarray(mask).astype(np.uint8)
    Wq = np.asarray(Wq, dtype=np.float32)
    Wv = np.asarray(Wv, dtype=np.float32)
    Wa = np.asarray(Wa, dtype=np.float32)
    bias = np.asarray(bias, dtype=np.float32)
    Wconv = np.asarray(Wconv, dtype=np.float32)
    Wloc = np.asarray(Wloc, dtype=np.float32)

    xpad = np.pad(law, ((0, 0), (0, 0), (PAD, PAD)))
    WvT = np.ascontiguousarray(Wv.T)                       # [256, 128]
    WqT = np.ascontiguousarray(Wq.T)                       # [512, 128]
    Wcomb = Wloc @ Wconv.reshape(FILT, CK)                 # [128, 62]
    WcT = np.ascontiguousarray(Wcomb.T)                    # [62, 128]
    Wa_col = np.ascontiguousarray(Wa[0][:, None])          # [128, 1]
    bias_col = np.ascontiguousarray(bias[:, None])         # [128, 1]
    # WaEmb[a, b*BL + j] = Wa[a] if j == b else 0
    waemb = np.zeros((ATT_DIM, BL * BL), dtype=np.float32)
    for b in range(BL):
        waemb[:, b * BL + b] = Wa[0]

    in_maps = []
    for i in range(NCORES):
        s = slice(i * BL, (i + 1) * BL)
        in_maps.append({
            "value": np.ascontiguousarray(value[s]),
            "xpad": np.ascontiguousarray(xpad[s]),
            "queryl": np.ascontiguousarray(query[s]),
            "maskl": np.ascontiguousarray(mask_u8[s]),
            "WvT": WvT, "WqT": WqT, "WcT": WcT,
            "Wa": Wa_col, "WaEmb": waemb, "biasq": bias_col,
        })
    return in_maps


def kernel(**inputs):
    nc = get_nc()
    in_maps = make_in_maps(**inputs)
    res = run_bass_kernel_spmd(nc, in_maps, core_ids=list(range(NCORES)))
    ctx = np.concatenate([r["out_ctx"] for r in res.results], axis=0)
    align = np.concatenate([r["out_align"] for r in res.results], axis=0)
    return ctx, align
